# revision 26
# baseline (speedup 1.0000x reference)
"""AttentionBlock (1x1-conv QKV attention, C=512, HW=32x32, B=32) on 8 TRN2 cores.

Strategy: pure data parallelism over batch — 4 images per core, no collectives.

Algebraic folding (host-side, weights only):
  s[n,m] = q_n . k_m = x_n^T (Wq^T Wk) x_m          -> A := Wq^T Wk  [C,C]
  out    = Wo (P (Wv x)) = (Wo Wv) x P^T            -> U := Wo Wv   [C,C]
so per image only FOUR device matmul stages remain (t = A x, zT = (U x)^T,
sT = t^T x, out = zT^T eT) instead of six, and no PE transposes at all:
scores are computed key-major (sT[m,n]) so exp(sT) is already in the layout
the attention*V contraction needs. Softmax row sums are obtained with an
all-ones stationary matmul over eT that broadcasts rs[n] to all 128 psum
partitions; normalization (1/rs) is applied at the final drain.

All big matmuls run in fp8(e4m3) DoubleRow perf mode (2 channel-planes per
pass): lhsT [K,2,M], rhs [K,2,N] slices of [P, tile, free] SBUF tensors.
Host pre-scales A and U by powers of two so fp8 tensors sit near unit std;
all compensating scales are powers of two, passed via a small const table
(per-partition scalars), and cancel exactly at the final drain.

Per image the PSUM->SBUF drains are balanced across engines (GpSimd cannot
read PSUM on TRN2, so all drains sit on Act/DVE):
  Act:    8 exp drains (EXP activation, scale AP, bias -ln 8) + 4 zT drains
  DVE:    4 t drains, row-sum reciprocal (approx-fast), 4 AV drains fused as
          (psum * 1/G) * inv via scalar_tensor_tensor
  GpSimd: 4 SBUF-only residual adds (y = u + x_bf16)
Residual uses bf16 x (rel err ~1.7e-3 of the dominant term, well within
gate); output written f32. The AV phase of image b-1 is software-pipelined
into the scores phase of image b so the PE stays >95% busy steady-state.

Biases: bv/bo are folded on the host into the residual term (x + Wo bv + bo);
bq/bk are structurally zero in this module's init — if nonzero inputs are ever
passed, kernel() falls back to a bf16 general-path kernel that applies them.
"""

import numpy as np

B = 32
C = 512
H = 32
W = 32
HW = H * W
N_CORES = 8
B_LOC = B // N_CORES  # 4 images per core
P = 128
CT = C // P  # 4 channel partition-tiles
NT = HW // P  # 8 hw partition-tiles
NC2 = HW // 512  # 2 free-dim chunks of 512
SCALE = float(C) ** -0.5
BEXP = -2.0794415416798357  # -ln(8): keeps exp() < 240 (fp8e4m3 max finite)

_NC_CACHE = {}


def _ts(i, size):
    return slice(i * size, (i + 1) * size)


def build_nc_fast():
    import concourse.bacc as bacc
    import concourse.mybir as mybir
    import concourse.tile as tile
    from contextlib import ExitStack

    F32 = mybir.dt.float32
    BF16 = mybir.dt.bfloat16
    FP8 = mybir.dt.float8e4
    EXP = mybir.ActivationFunctionType.Exp
    DR = mybir.MatmulPerfMode.DoubleRow
    MULT = mybir.AluOpType.mult
    ADD = mybir.AluOpType.add

    nc = bacc.Bacc()
    x16_ext = nc.declare_dram_parameter("x16", [B_LOC, C, HW], BF16, isOutput=False)
    x8_ext = nc.declare_dram_parameter("x8", [B_LOC, C, HW], FP8, isOutput=False)
    a_ext = nc.declare_dram_parameter("amat", [C, C], FP8, isOutput=False)
    u_ext = nc.declare_dram_parameter("umat", [C, C], FP8, isOutput=False)
    ones_ext = nc.declare_dram_parameter("ones8", [P, 2, P], FP8, isOutput=False)
    c_ext = nc.declare_dram_parameter("consts", [P, 8], F32, isOutput=False)
    out_ext = nc.declare_dram_parameter("out", [B_LOC, C, HW], F32, isOutput=True)

    with tile.TileContext(nc) as tc, ExitStack() as ctx:
        singles = ctx.enter_context(tc.tile_pool(name="singles", bufs=1))
        x16pool = ctx.enter_context(tc.tile_pool(name="x16pool", bufs=2))
        x8pool = ctx.enter_context(tc.tile_pool(name="x8pool", bufs=2))
        tpool = ctx.enter_context(tc.tile_pool(name="tpool", bufs=2))
        ztpool = ctx.enter_context(tc.tile_pool(name="ztpool", bufs=2))
        etpool = ctx.enter_context(tc.tile_pool(name="etpool", bufs=2))
        invpool = ctx.enter_context(tc.tile_pool(name="invpool", bufs=2))
        upool = ctx.enter_context(tc.tile_pool(name="upool", bufs=2))
        ypool = ctx.enter_context(tc.tile_pool(name="ypool", bufs=3))
        psA = ctx.enter_context(tc.tile_pool(name="psA", bufs=3, space="PSUM"))
        psR = ctx.enter_context(tc.tile_pool(name="psR", bufs=1, space="PSUM"))

        a_sb = singles.tile([P, CT, C], FP8)
        u_sb = singles.tile([P, CT, C], FP8)
        ones_sb = singles.tile([P, 2, P], FP8)
        c_sb = singles.tile([P, 8], F32)
        warm = singles.tile([P, P], BF16)
        wact = singles.tile([P, 1], F32)
        nc.vector.memset(warm, 0.0)
        # amat first: the first real matmuls (t = A x) need it; consts next
        # (first drain reads them); ones/umat follow (needed later).
        nc.sync.dma_start(out=a_sb, in_=a_ext.rearrange("(t p) o -> p t o", p=P))
        nc.sync.dma_start(out=c_sb, in_=c_ext[:, :])
        nc.sync.dma_start(out=u_sb, in_=u_ext.rearrange("(t p) o -> p t o", p=P))
        nc.sync.dma_start(out=ones_sb, in_=ones_ext[:, :, :])
        # Prime the Act engine's Exp table now (1.28us load) so the first
        # real exp drain doesn't pay it mid-pipeline.
        nc.scalar.activation(wact, warm[:, 0:1], EXP, bias=c_sb[:, 5:6])
        # Warm up the PE (HAM clock gate) with throwaway matmuls while the
        # first DMAs are in flight, so real matmuls start at full clock.
        # Few enough that the queue is clear when the first image lands
        # (each stretches to ~330ns from psum-pool semaphore rotation).
        for _w in range(12):
            wps = psA.tile([P, HW], F32, tag="ps")
            nc.tensor.matmul(wps[:, 0:P], lhsT=warm, rhs=warm)

        # Per-image state threaded through the software pipeline: the AV
        # phase of image b-1 is interleaved into the scores phase of image b
        # so the PE never waits for the Act-bound exp drains alone.
        state = {}

        def emit_av_group(st, cb):
            ps = psA.tile([P, HW], F32, tag="ps")
            for nch in range(2):
                for mp in range(4):
                    nc.tensor.matmul(
                        ps[:, _ts(nch, 512)],
                        lhsT=st["zt"][:, 2 * mp : 2 * mp + 2, _ts(cb, P)],
                        rhs=st["et"][:, 2 * mp : 2 * mp + 2, _ts(nch, 512)],
                        start=(mp == 0),
                        stop=(mp == 3),
                        perf_mode=DR,
                    )
            u_t = upool.tile([P, HW], F32, tag="u")
            nc.vector.scalar_tensor_tensor(
                u_t, ps, c_sb[:, 3:4], st["inv"], MULT, MULT
            )
            y_t = ypool.tile([P, HW], F32, tag="y")
            # alternate the residual add between GpSimd and DVE: halves the
            # serial GpSimd chain that dominates the last image's tail
            if cb % 2 == 0:
                nc.gpsimd.tensor_add(y_t, u_t, st["x16"][:, cb, :])
            else:
                nc.vector.tensor_add(y_t, u_t, st["x16"][:, cb, :])
            nc.sync.dma_start(out=st["yr"][:, cb, :], in_=y_t)

        for b in range(B_LOC):
            x8_sb = x8pool.tile([P, CT, HW], FP8)
            x16_sb = x16pool.tile([P, CT, HW], BF16)
            x8r = x8_ext[b].rearrange("(t p) m -> p t m", p=P)
            x16r = x16_ext[b].rearrange("(t p) m -> p t m", p=P)
            # one trigger for the whole image: the Sync engine serializes
            # DMA triggers at ~650ns each, and the t groups need all four
            # channel-tiles anyway, so per-chunk loads only delay startup
            nc.sync.dma_start(out=x8_sb, in_=x8r[:, :, :])

            # --- t = A x [c, m] and zT = (U x)^T [m, c]. t groups are
            # front-loaded (t0 t1 z0 t2 z1 t3 z2 z3) so the last DVE t-drain
            # lands before the scores matmuls need it, while the Act-drained
            # zT groups still overlap the DVE t-drains ---------------------
            t_sb = tpool.tile([P, CT, HW], FP8)
            zt_sb = ztpool.tile([P, NT, C], FP8)

            def emit_t_group(cb):
                ps = psA.tile([P, HW], F32, tag="ps")
                for nch in range(2):
                    for pr in range(2):
                        nc.tensor.matmul(
                            ps[:, _ts(nch, 512)],
                            lhsT=a_sb[:, 2 * pr : 2 * pr + 2, _ts(cb, P)],
                            rhs=x8_sb[:, 2 * pr : 2 * pr + 2, _ts(nch, 512)],
                            start=(pr == 0),
                            stop=(pr == 1),
                            perf_mode=DR,
                        )
                nc.vector.tensor_scalar_mul(t_sb[:, cb, :], ps, c_sb[:, 0:1])

            def emit_zt_group(g):
                ps = psA.tile([P, HW], F32, tag="ps")
                for half in range(2):
                    mb = 2 * g + half
                    for pr in range(2):
                        nc.tensor.matmul(
                            ps[:, _ts(half, 512)],
                            lhsT=x8_sb[:, 2 * pr : 2 * pr + 2, _ts(mb, P)],
                            rhs=u_sb[:, 2 * pr : 2 * pr + 2, :],
                            start=(pr == 0),
                            stop=(pr == 1),
                            perf_mode=DR,
                        )
                dst = zt_sb[:, 2 * g : 2 * g + 2, :].rearrange("p a c -> p (a c)")
                nc.scalar.mul(dst, ps, c_sb[:, 2:3])

            if b == 0:
                # no previous-image AV work overlaps image 0's phases, so
                # emit all t groups first: their DVE drains (which gate the
                # first scores matmuls) finish while the PE runs zT groups
                order = (("t", 0), ("t", 1), ("t", 2), ("t", 3),
                         ("z", 0), ("z", 1), ("z", 2), ("z", 3))
            else:
                # later images: stagger so t-drains (DVE) interleave with the
                # previous image's AV drains already queued on DVE
                order = (("t", 0), ("t", 1), ("z", 0), ("t", 2),
                         ("z", 1), ("t", 3), ("z", 2), ("z", 3))
            for kind, idx in order:
                if kind == "t":
                    emit_t_group(idx)
                else:
                    emit_zt_group(idx)

            # x16 is first read by this image's AV drains (which run inside
            # the NEXT image's scores phase) — trigger its load late so the
            # early DMA bandwidth goes to x8/amat/umat instead.
            nc.sync.dma_start(out=x16_sb, in_=x16r[:, :, :])

            # --- sT = t^T x ; eT = exp(sT/sqrt(C) - ln8); rs via ones.
            # AV groups of the previous image ride along here -------------
            et_sb = etpool.tile([P, NT, HW], FP8)
            rs_ps = psR.tile([P, HW], F32, tag="rs")
            for mb in range(NT):
                ps = psA.tile([P, HW], F32, tag="ps")
                for nch in range(2):
                    for pr in range(2):
                        nc.tensor.matmul(
                            ps[:, _ts(nch, 512)],
                            lhsT=t_sb[:, 2 * pr : 2 * pr + 2, _ts(mb, P)],
                            rhs=x8_sb[:, 2 * pr : 2 * pr + 2, _ts(nch, 512)],
                            start=(pr == 0),
                            stop=(pr == 1),
                            perf_mode=DR,
                        )
                nc.scalar.activation(
                    et_sb[:, mb, :], ps, EXP, bias=c_sb[:, 4:5], scale=c_sb[:, 1:2]
                )
                if mb % 2 == 1:
                    mp = mb // 2
                    for nch in range(2):
                        nc.tensor.matmul(
                            rs_ps[:, _ts(nch, 512)],
                            lhsT=ones_sb,
                            rhs=et_sb[:, mb - 1 : mb + 1, _ts(nch, 512)],
                            start=(mp == 0),
                            stop=(mp == 3),
                            perf_mode=DR,
                        )
                    if state:
                        emit_av_group(state, mb // 2)
            inv_sb = invpool.tile([P, HW], F32)
            nc.vector.reciprocal_approx_fast(out=inv_sb, in_=rs_ps)

            state = {
                "zt": zt_sb,
                "et": et_sb,
                "inv": inv_sb,
                "x16": x16_sb,
                "yr": out_ext[b].rearrange("(t p) m -> p t m", p=P),
            }

        # drain the last image's AV phase
        for cb in range(CT):
            emit_av_group(state, cb)

    nc.compile()
    return nc


def make_in_maps_fast(x, Wq, bq, Wk, bk, Wv, bv, Wo, bo):
    import ml_dtypes

    FP8 = ml_dtypes.float8_e4m3
    BF16 = ml_dtypes.bfloat16
    f64 = np.float64

    x = np.asarray(x, dtype=np.float32).reshape(B, C, HW)
    std_x = float(x.std()) or 1.0

    # A^T layout [c2, c1] for t = A x;  A = Wq^T Wk
    At = np.asarray(Wk, f64).T @ np.asarray(Wq, f64)
    # U^T layout [c', c] for zT = x^T U^T;  U = Wo Wv
    Ut = np.asarray(Wv, f64).T @ np.asarray(Wo, f64).T

    def pow2(v):
        return float(2.0 ** np.round(np.log2(v)))

    std_A = float(At.std()) or 1.0
    s_A = pow2(1.0 / std_A)  # fp8 A entries ~ unit std
    # t psum std ~ s_A*std_A*sqrt(C)*std_x; target t_sb std ~ 1.2
    d_t = pow2(1.2 / (s_A * std_A * np.sqrt(C) * std_x))
    lam_t = s_A * d_t  # t_sb = lam_t * t_true
    exp_scale = float(SCALE / (lam_t * std_x**0))  # s_psum = lam_t * s_raw

    std_U = float(Ut.std()) or 1.0
    s_U = pow2(1.0 / std_U)  # fp8 U entries ~ unit std
    # zT psum std ~ s_U*std_U*sqrt(C)*std_x; target zT_sb std ~ 2.0
    d_z = pow2(2.0 / (s_U * std_U * np.sqrt(C) * std_x))
    G = s_U * d_z  # zT_sb = G * z_true

    consts = np.zeros((P, 8), dtype=np.float32)
    consts[:, 0] = d_t
    consts[:, 1] = exp_scale
    consts[:, 2] = d_z
    consts[:, 3] = 1.0 / G
    consts[:, 4] = BEXP

    amat = np.ascontiguousarray(At * s_A).astype(FP8)
    umat = np.ascontiguousarray(Ut * s_U).astype(FP8)
    ones8 = np.ones((P, 2, P), dtype=FP8)

    # residual carries the folded output biases: y = x + Wo bv + bo + atten
    bres = (
        np.asarray(Wo, f64) @ np.asarray(bv, f64) + np.asarray(bo, f64)
    ).astype(np.float32)
    x16 = (x + bres[None, :, None]).astype(BF16)
    x8 = x.astype(FP8)

    return [
        {
            "x16": np.ascontiguousarray(x16[i * B_LOC : (i + 1) * B_LOC]),
            "x8": np.ascontiguousarray(x8[i * B_LOC : (i + 1) * B_LOC]),
            "amat": amat,
            "umat": umat,
            "ones8": ones8,
            "consts": consts,
        }
        for i in range(N_CORES)
    ]


# ---------------------------------------------------------------------------
# General fallback path (bf16, handles arbitrary bq/bk), from the baseline.
# ---------------------------------------------------------------------------


def build_nc_general():
    import concourse.bacc as bacc
    import concourse.mybir as mybir
    import concourse.tile as tile
    from concourse.masks import make_identity
    from contextlib import ExitStack

    F32 = mybir.dt.float32
    BF16 = mybir.dt.bfloat16
    EXP = mybir.ActivationFunctionType.Exp
    IDENT = mybir.ActivationFunctionType.Identity

    nc = bacc.Bacc()
    x_ext = nc.declare_dram_parameter("x", [B_LOC, C, HW], F32, isOutput=False)
    xb_ext = nc.declare_dram_parameter("xb", [B_LOC, C, HW], BF16, isOutput=False)
    wq_ext = nc.declare_dram_parameter("wq", [C, C], BF16, isOutput=False)
    wk_ext = nc.declare_dram_parameter("wk", [C, C], BF16, isOutput=False)
    wv_ext = nc.declare_dram_parameter("wv", [C, C], BF16, isOutput=False)
    wo_ext = nc.declare_dram_parameter("wo", [C, C], BF16, isOutput=False)
    bias_ext = nc.declare_dram_parameter("bias", [P, 16], F32, isOutput=False)
    out_ext = nc.declare_dram_parameter("out", [B_LOC, C, HW], F32, isOutput=True)

    with tile.TileContext(nc) as tc, ExitStack() as ctx:
        singles = ctx.enter_context(tc.tile_pool(name="singles", bufs=1))
        xpool = ctx.enter_context(tc.tile_pool(name="xpool", bufs=2))
        xbpool = ctx.enter_context(tc.tile_pool(name="xbpool", bufs=2))
        qkpool = ctx.enter_context(tc.tile_pool(name="qkpool", bufs=2))
        vtpool = ctx.enter_context(tc.tile_pool(name="vtpool", bufs=2))
        epool = ctx.enter_context(tc.tile_pool(name="epool", bufs=3))
        ptpool = ctx.enter_context(tc.tile_pool(name="ptpool", bufs=1))
        htpool = ctx.enter_context(tc.tile_pool(name="htpool", bufs=1))
        ypool = ctx.enter_context(tc.tile_pool(name="ypool", bufs=2))
        smpool = ctx.enter_context(tc.tile_pool(name="smpool", bufs=4))
        psmm = ctx.enter_context(tc.tile_pool(name="psmm", bufs=3, space="PSUM"))
        pstr = ctx.enter_context(tc.tile_pool(name="pstr", bufs=2, space="PSUM"))

        wq_sb = singles.tile([P, CT, C], BF16)
        wk_sb = singles.tile([P, CT, C], BF16)
        wv_sb = singles.tile([P, CT, C], BF16)
        wo_sb = singles.tile([P, CT, C], BF16)
        bias_sb = singles.tile([P, 16], F32)
        ident = singles.tile([P, P], BF16)
        make_identity(nc, ident)
        nc.sync.dma_start(out=bias_sb, in_=bias_ext[:, :])
        nc.sync.dma_start(out=wq_sb, in_=wq_ext.rearrange("(t p) o -> p t o", p=P))
        nc.sync.dma_start(out=wk_sb, in_=wk_ext.rearrange("(t p) o -> p t o", p=P))
        nc.sync.dma_start(out=wv_sb, in_=wv_ext.rearrange("(t p) o -> p t o", p=P))
        nc.sync.dma_start(out=wo_sb, in_=wo_ext.rearrange("(t p) o -> p t o", p=P))
        for _w in range(48):
            wps = pstr.tile([P, 4, P], F32, tag="pt")
            nc.tensor.matmul(wps[:, 0, :], lhsT=ident, rhs=ident)

        for b in range(B_LOC):
            xb_sb = xbpool.tile([P, CT, HW], BF16)
            x_sb = xpool.tile([P, CT, HW], F32)
            xr = x_ext[b].rearrange("(t p) m -> p t m", p=P)
            xbr = xb_ext[b].rearrange("(t p) m -> p t m", p=P)
            for c_t in range(CT):
                nc.sync.dma_start(out=xb_sb[:, c_t, :], in_=xbr[:, c_t, :])
            for c_t in range(CT):
                nc.sync.dma_start(out=x_sb[:, c_t, :], in_=xr[:, c_t, :])

            q_sb = qkpool.tile([P, CT, HW], BF16, tag="q")
            k_sb = qkpool.tile([P, CT, HW], BF16, tag="k")
            for co_t in range(CT):
                psq = psmm.tile([P, HW], F32, tag="ps")
                for ncx in range(NC2):
                    for ci_t in range(CT):
                        nc.tensor.matmul(
                            psq[:, _ts(ncx, 512)],
                            lhsT=wq_sb[:, ci_t, _ts(co_t, P)],
                            rhs=xb_sb[:, ci_t, _ts(ncx, 512)],
                            start=(ci_t == 0),
                            stop=(ci_t == CT - 1),
                        )
                nc.scalar.activation(
                    q_sb[:, co_t, :], psq, IDENT,
                    bias=bias_sb[:, 0 + co_t : 1 + co_t],
                )
                psk = psmm.tile([P, HW], F32, tag="ps")
                for ncx in range(NC2):
                    for ci_t in range(CT):
                        nc.tensor.matmul(
                            psk[:, _ts(ncx, 512)],
                            lhsT=wk_sb[:, ci_t, _ts(co_t, P)],
                            rhs=xb_sb[:, ci_t, _ts(ncx, 512)],
                            start=(ci_t == 0),
                            stop=(ci_t == CT - 1),
                        )
                nc.scalar.activation(
                    k_sb[:, co_t, :], psk, IDENT,
                    bias=bias_sb[:, 4 + co_t : 5 + co_t],
                )

            vt_sb = vtpool.tile([P, NT, C], BF16)
            for m_t in range(0, NT, 2):
                psv = psmm.tile([P, HW], F32, tag="ps")
                for half in range(2):
                    for ci_t in range(CT):
                        nc.tensor.matmul(
                            psv[:, _ts(half, 512)],
                            lhsT=xb_sb[:, ci_t, _ts(m_t + half, P)],
                            rhs=wv_sb[:, ci_t, :],
                            start=(ci_t == 0),
                            stop=(ci_t == CT - 1),
                        )
                nc.vector.tensor_copy(
                    vt_sb[:, m_t : m_t + 2, :].rearrange("p a c -> p (a c)"), psv
                )

            pt_sb = ptpool.tile([P, NT, HW], BF16)
            for n_t in range(NT):
                e_t = epool.tile([P, HW], BF16, tag="e")
                rs = smpool.tile([P, 1], F32, tag="rs")
                pss = psmm.tile([P, HW], F32, tag="ps")
                for mcx in range(NC2):
                    for c_t in range(CT):
                        nc.tensor.matmul(
                            pss[:, _ts(mcx, 512)],
                            lhsT=q_sb[:, c_t, _ts(n_t, P)],
                            rhs=k_sb[:, c_t, _ts(mcx, 512)],
                            start=(c_t == 0),
                            stop=(c_t == CT - 1),
                        )
                nc.scalar.activation(
                    e_t, pss, EXP, scale=SCALE, accum_out=rs,
                )
                inv = smpool.tile([P, 1], F32, tag="inv")
                nc.vector.reciprocal(inv, rs)
                dmat = smpool.tile([P, P], BF16, tag="dmat")
                nc.vector.tensor_scalar_mul(dmat, ident, inv)
                for grp in range(2):
                    pst = pstr.tile([P, 4, P], F32, tag="pt")
                    for j in range(4):
                        m_t = grp * 4 + j
                        nc.tensor.matmul(
                            pst[:, j, :], lhsT=e_t[:, _ts(m_t, P)], rhs=dmat
                        )
                    dst = pt_sb[:, grp * 4 : grp * 4 + 4, _ts(n_t, P)]
                    if grp == 0:
                        nc.vector.tensor_copy(dst, pst)
                    else:
                        nc.scalar.copy(dst, pst)

            ht_sb = htpool.tile([P, CT, HW], BF16)
            for c_t in range(CT):
                psh = psmm.tile([P, HW], F32, tag="ps")
                for ncx in range(NC2):
                    for m_t in range(NT):
                        nc.tensor.matmul(
                            psh[:, _ts(ncx, 512)],
                            lhsT=vt_sb[:, m_t, _ts(c_t, P)],
                            rhs=pt_sb[:, m_t, _ts(ncx, 512)],
                            start=(m_t == 0),
                            stop=(m_t == NT - 1),
                        )
                nc.scalar.activation(
                    ht_sb[:, c_t, :], psh, IDENT,
                    bias=bias_sb[:, 8 + c_t : 9 + c_t],
                )

            y_sb = ypool.tile([P, CT, HW], F32)
            yr = out_ext[b].rearrange("(t p) m -> p t m", p=P)
            for co_t in range(CT):
                pso = psmm.tile([P, HW], F32, tag="ps")
                for ncx in range(NC2):
                    for c_t in range(CT):
                        nc.tensor.matmul(
                            pso[:, _ts(ncx, 512)],
                            lhsT=wo_sb[:, c_t, _ts(co_t, P)],
                            rhs=ht_sb[:, c_t, _ts(ncx, 512)],
                            start=(c_t == 0),
                            stop=(c_t == CT - 1),
                        )
                h2 = smpool.tile([P, HW], F32, tag="h2")
                nc.vector.tensor_scalar_add(
                    h2, pso, bias_sb[:, 12 + co_t : 13 + co_t]
                )
                nc.vector.tensor_add(y_sb[:, co_t, :], h2, x_sb[:, co_t, :])
                nc.sync.dma_start(out=yr[:, co_t, :], in_=y_sb[:, co_t, :])

    nc.compile()
    return nc


def make_in_maps_general(x, Wq, bq, Wk, bk, Wv, bv, Wo, bo):
    import ml_dtypes

    x = np.asarray(x, dtype=np.float32).reshape(B, C, HW)
    xb = x.astype(ml_dtypes.bfloat16)
    wqT = np.ascontiguousarray(np.asarray(Wq, dtype=np.float32).T).astype(
        ml_dtypes.bfloat16
    )
    wkT = np.ascontiguousarray(np.asarray(Wk, dtype=np.float32).T).astype(
        ml_dtypes.bfloat16
    )
    wvT = np.ascontiguousarray(np.asarray(Wv, dtype=np.float32).T).astype(
        ml_dtypes.bfloat16
    )
    woT = np.ascontiguousarray(np.asarray(Wo, dtype=np.float32).T).astype(
        ml_dtypes.bfloat16
    )
    bias = np.zeros((P, 16), dtype=np.float32)
    for i, bvec in enumerate([bq, bk, bv, bo]):
        bias[:, i * 4 : (i + 1) * 4] = (
            np.asarray(bvec, dtype=np.float32).reshape(CT, P).T
        )
    return [
        {
            "x": np.ascontiguousarray(x[i * B_LOC : (i + 1) * B_LOC]),
            "xb": np.ascontiguousarray(xb[i * B_LOC : (i + 1) * B_LOC]),
            "wq": wqT,
            "wk": wkT,
            "wv": wvT,
            "wo": woT,
            "bias": bias,
        }
        for i in range(N_CORES)
    ]


def _get_nc(fast=True):
    key = "fast" if fast else "general"
    if key not in _NC_CACHE:
        _NC_CACHE[key] = build_nc_fast() if fast else build_nc_general()
    return _NC_CACHE[key]


def _use_fast(bq, bk):
    return not (np.any(np.asarray(bq)) or np.any(np.asarray(bk)))


def make_in_maps(x, Wq, bq, Wk, bk, Wv, bv, Wo, bo):
    if _use_fast(bq, bk):
        return make_in_maps_fast(x, Wq, bq, Wk, bk, Wv, bv, Wo, bo)
    return make_in_maps_general(x, Wq, bq, Wk, bk, Wv, bv, Wo, bo)


def kernel(x, Wq, bq, Wk, bk, Wv, bv, Wo, bo):
    from concourse.bass_utils import run_bass_kernel_spmd

    fast = _use_fast(bq, bk)
    nc = _get_nc(fast)
    if fast:
        in_maps = make_in_maps_fast(x, Wq, bq, Wk, bk, Wv, bv, Wo, bo)
    else:
        in_maps = make_in_maps_general(x, Wq, bq, Wk, bk, Wv, bv, Wo, bo)
    res = run_bass_kernel_spmd(nc, in_maps, core_ids=list(range(N_CORES)))
    out = np.concatenate([res.results[i]["out"] for i in range(N_CORES)], axis=0)
    return out.reshape(B, C, H, W).astype(np.float32)
